# revision 17
# baseline (speedup 1.0000x reference)
"""Trainium2 Bass kernel for GaussMonom: out[n] = const * exp(-(x[n]-mean) @ cov @ (x[n]-mean)).

Strategy (memory-bound, trivially data-parallel):
  - Shard the N=16.7M points across 8 cores (2,097,152 points/core).
  - Per core, view the [per, 2] slab as [128, 32768] f32 (row-major), so each
    partition row holds 16384 points with (x0, x1) interleaved. Loads are fully
    contiguous per partition; x0/x1 are read on-chip via stride-2 APs.
  - Host-side, expand zeta to a polynomial in (x0, x1) and complete the square
    in x0 (absorbing the cross term) and then in x1:
        zeta = a*(x0 + alpha*x1 + beta)^2 + c'*(x1 + gamma)^2 + g''
    so per tile the work is 1 DVE STT (u = alpha*x1 + x0), 2 ACT Squares (the
    bias absorbs beta/gamma), 1 DVE STT combine, and 1 ACT Exp that absorbs
    the scale by -a, the constant g'', and ln(const). No Pool/GPSIMD pass.
  - The output is stored as bf16 (max rel err ~2e-3, well under the 2e-2
    gate) and widened to fp32 on the host: per-core HBM traffic drops from
    16+8 MiB to 16+4 MiB, which is the bottleneck (360 GB/s DMA).
"""

import math

import numpy as np

try:
    from concourse import bacc, bass, mybir, tile
    from concourse import bass_utils
except ImportError:  # path fallback for bare containers
    import sys

    sys.path.insert(0, "/opt/trn_rl_repo")
    from concourse import bacc, bass, mybir, tile
    from concourse import bass_utils

N_CORES = 8
P = 128  # SBUF partitions
CW = 4096  # input columns per tile (2 MiB loads)

# Toggled by test.py for profiling; harness uses the defaults.
TRACE = False
TRACE_KWARGS = {}
LAST_RESULTS = None

FP32 = mybir.dt.float32
BF16 = mybir.dt.bfloat16
MULT = mybir.AluOpType.mult
ADD = mybir.AluOpType.add
SQUARE = mybir.ActivationFunctionType.Square
EXP = mybir.ActivationFunctionType.Exp


def _tile_plan(W, CW):
    """Column offsets/widths: uniform CW tiles, with the last CW-wide chunk
    tapered (2048,1024,512,512) so the tail's compute-chain latency shrinks."""
    taper = [CW // 2, CW // 4, CW // 8, CW // 8]
    plan = []
    off = 0
    for _ in range(W // CW - 1):
        plan.append((off, CW))
        off += CW
    for s in taper:
        plan.append((off, s))
        off += s
    assert off == W
    return plan


def _tile_plan_uniform(W, CW):
    return [(off, CW) for off in range(0, W, CW)]


def _emit_fast(nc, x, y, W, CW, co):
    """zeta = a*(x0 + alpha*x1 + beta)^2 + c'*(x1 + gamma)^2 + g''
    u = alpha*x1 + x0; A1 = (u+beta)^2; A2 = (x1+gamma)^2;
    z = A1 + (c'/a)*A2; out = exp(-a*z + (-g'' + ln K)) as bf16.
    Requires a != 0, c' != 0, K > 0.

    Engine budget per full tile (F=2048 pts/partition): ACT 3 passes
    (Square, Square, Exp ~5.7us), DVE 2 STT passes (~4.4us), vs 5.8us of
    load DMA — memory-bound. All outputs accumulate in SBUF (32 KiB/
    partition total) and are stored by just three big deferred DMAs on the
    sync queue, emitted after the last load: the loads run back-to-back on
    the (FIFO) DMA engines, and the region stores slot in right behind
    them with zero idle. Region boundaries (tiles 0..n-5 | n-4..n-2 |
    n-1) are chosen so each region's Exps have long finished by the time
    its store's turn on the DMA engines comes up. Software pipelining
    (z/Exp for tile k emitted after tile k+1's squares) keeps the in-order
    ACT engine from head-of-line blocking on DVE round trips."""
    plan = _tile_plan_uniform(W, CW)
    n = len(plan)
    assert n >= 5
    Wout = W // 2
    # Output-region split: [0, s1) | [s1, s2) | [s2, n) in tile indices.
    # regA's store is granted right after the last load; its Exps (and then
    # regB's, regC's) must each be done before the previous region's store
    # finishes on the FIFO DMA engines.
    s1, s2 = n - 4, n - 1
    bounds = [plan[s1][0] // 2, plan[s2][0] // 2, Wout]
    with tile.TileContext(nc) as tc:
        with (
            tc.tile_pool(name="cst", bufs=1) as cst_pool,
            tc.tile_pool(name="xin", bufs=3) as xin_pool,
            tc.tile_pool(name="upre", bufs=2) as upre_pool,
            tc.tile_pool(name="sq", bufs=4) as sq_pool,
            tc.tile_pool(name="zz", bufs=3) as z_pool,
            tc.tile_pool(name="reg", bufs=1) as reg_pool,
        ):
            cb_beta = cst_pool.tile([P, 1], FP32, tag="cb_beta")
            nc.gpsimd.memset(cb_beta[:], co["beta"])
            cb_gamma = cst_pool.tile([P, 1], FP32, tag="cb_gamma")
            nc.gpsimd.memset(cb_gamma[:], co["gamma"])
            cb_e = cst_pool.tile([P, 1], FP32, tag="cb_e")
            nc.gpsimd.memset(cb_e[:], co["bias_e"])

            # Warm the ACT function tables on [P,1] dummies while the first
            # load is still in flight, so the one-time table-load cost never
            # lands in the ACT stream.
            warm = cst_pool.tile([P, 1], FP32, tag="warm")
            nc.scalar.activation(warm[:], cb_beta[:], SQUARE, bias=0.0, scale=1.0)
            nc.scalar.activation(warm[:], cb_beta[:], EXP, bias=0.0, scale=1.0)

            reg_a = reg_pool.tile([P, bounds[0]], BF16, tag="regA")
            reg_b = reg_pool.tile([P, bounds[1] - bounds[0]], BF16, tag="regB")
            reg_c = reg_pool.tile([P, Wout - bounds[1]], BF16, tag="regC")
            regs = [reg_a, reg_b, reg_c]

            def out_slice(k):
                off, cw = plan[k]
                o0, o1 = off // 2, off // 2 + cw // 2
                r = 0 if k < s1 else (1 if k < s2 else 2)
                base = 0 if r == 0 else bounds[r - 1]
                return regs[r][:, o0 - base : o1 - base]

            def stage1(k):
                off, cw = plan[k]
                F = cw // 2
                xt = xin_pool.tile([P, cw], FP32, tag="xt")
                nc.sync.dma_start(xt[:], x[:, off : off + cw])
                x0 = xt[:, 0::2]
                x1 = xt[:, 1::2]
                # A2 only needs the load; it runs on ACT while DVE makes u.
                a2 = sq_pool.tile([P, F], FP32, tag="a2")
                nc.scalar.activation(a2[:], x1, SQUARE, bias=cb_gamma[:], scale=1.0)
                up = upre_pool.tile([P, F], FP32, tag="up")
                nc.vector.scalar_tensor_tensor(up[:], x1, co["alpha"], x0, MULT, ADD)
                a1 = sq_pool.tile([P, F], FP32, tag="a1")
                nc.scalar.activation(a1[:], up[:], SQUARE, bias=cb_beta[:], scale=1.0)
                return a1, a2

            def stage2(k, a1, a2):
                F = plan[k][1] // 2
                z = z_pool.tile([P, F], FP32, tag="z")
                nc.vector.scalar_tensor_tensor(z[:], a2[:], co["r"], a1[:], MULT, ADD)
                nc.scalar.activation(
                    out_slice(k), z[:], EXP, bias=cb_e[:], scale=co["neg_a"]
                )

            prev = stage1(0)
            for k in range(1, n):
                cur = stage1(k)
                stage2(k - 1, *prev)
                prev = cur
            stage2(n - 1, *prev)

            # Deferred region stores, in readiness order right behind the
            # final load on the FIFO DMA engines.
            lo = 0
            for r in range(3):
                nc.sync.dma_start(y[:, lo : bounds[r]], regs[r][:])
                lo = bounds[r]


def _emit_general(nc, x, y, W, CW, co):
    """Fallback for degenerate coefficients: direct evaluation, more passes."""
    F = CW // 2
    ntiles = W // CW
    with tile.TileContext(nc) as tc:
        with (
            tc.tile_pool(name="xin", bufs=3) as xin_pool,
            tc.tile_pool(name="tmp", bufs=2) as tmp_pool,
            tc.tile_pool(name="oot", bufs=3) as out_pool,
        ):
            for i in range(ntiles):
                xt = xin_pool.tile([P, CW], FP32)
                nc.sync.dma_start(xt[:], x[:, i * CW : (i + 1) * CW])
                x0 = xt[:, 0::2]
                x1 = xt[:, 1::2]

                d0 = tmp_pool.tile([P, F], FP32)
                nc.vector.tensor_scalar_add(d0[:], x0, -co["m0"])
                d1 = tmp_pool.tile([P, F], FP32)
                nc.vector.tensor_scalar_add(d1[:], x1, -co["m1"])
                s1 = tmp_pool.tile([P, F], FP32)
                nc.scalar.mul(s1[:], d0[:], co["a"])
                s2 = tmp_pool.tile([P, F], FP32)
                nc.vector.scalar_tensor_tensor(s2[:], d1[:], co["b"], s1[:], MULT, ADD)
                s3 = tmp_pool.tile([P, F], FP32)
                nc.vector.tensor_mul(s3[:], s2[:], d0[:])
                s4 = tmp_pool.tile([P, F], FP32)
                nc.vector.scalar_tensor_tensor(s4[:], d1[:], co["c"], d1[:], MULT, MULT)
                s5 = tmp_pool.tile([P, F], FP32)
                nc.vector.tensor_add(s5[:], s3[:], s4[:])
                e = tmp_pool.tile([P, F], FP32)
                nc.scalar.activation(e[:], s5[:], EXP, bias=0.0, scale=-1.0)
                o = out_pool.tile([P, F], FP32)
                nc.vector.tensor_scalar_mul(o[:], e[:], co["K"])
                nc.sync.dma_start(y[:, i * F : (i + 1) * F], o[:])


def _coefficients(mean, cov, const):
    m0, m1 = float(mean[0]), float(mean[1])
    a = float(cov[0, 0])
    b = float(cov[0, 1]) + float(cov[1, 0])
    c = float(cov[1, 1])
    K = float(const[0])
    # zeta = a x0^2 + b x0 x1 + c x1^2 + e x0 + f x1 + g
    e = -(2.0 * a * m0 + b * m1)
    f = -(b * m0 + 2.0 * c * m1)
    g = a * m0 * m0 + b * m0 * m1 + c * m1 * m1

    co = {"m0": m0, "m1": m1, "a": a, "b": b, "c": c, "K": K}
    fast = abs(a) > 1e-30 and K > 0.0
    if fast:
        cp = c - b * b / (4.0 * a)  # c' after absorbing the cross term into x0
        fast = abs(cp) > 1e-30
    if fast:
        alpha = b / (2.0 * a)
        beta = e / (2.0 * a)
        fp = f - b * e / (2.0 * a)
        gamma = fp / (2.0 * cp)
        gpp = g - e * e / (4.0 * a) - cp * gamma * gamma
        co.update(
            alpha=alpha,
            beta=beta,
            gamma=gamma,
            r=cp / a,
            neg_a=-a,
            bias_e=-gpp + math.log(K),
        )
    return fast, co


_NC_CACHE = {}


def _build_cached(W, CW, fast, co):
    key = (W, CW, fast) + tuple(sorted(co.items()))
    nc = _NC_CACHE.get(key)
    if nc is None:
        nc = _build(W, CW, fast, co)
        _NC_CACHE[key] = nc
    return nc


def _build(W, CW, fast, co):
    nc = bacc.Bacc(
        "TRN2",
        target_bir_lowering=False,
        debug=False,
        enable_asserts=False,
        num_devices=N_CORES,
    )
    x = nc.dram_tensor("x", [P, W], FP32, kind="ExternalInput").ap()
    y_dt = BF16 if fast else FP32
    y = nc.dram_tensor("y", [P, W // 2], y_dt, kind="ExternalOutput").ap()
    if fast:
        _emit_fast(nc, x, y, W, CW, co)
    else:
        _emit_general(nc, x, y, W, CW, co)
    nc.compile()
    return nc


def kernel(tensor, mean, cov, const):
    global LAST_RESULTS
    tensor = np.ascontiguousarray(tensor, dtype=np.float32)
    mean = np.asarray(mean, dtype=np.float32)
    cov = np.asarray(cov, dtype=np.float32)
    const = np.asarray(const, dtype=np.float32)

    n = tensor.shape[0]
    per = n // N_CORES
    W = per * 2 // P  # f32 elements per partition row, per core
    assert n % N_CORES == 0 and (per * 2) % P == 0 and W % CW == 0, (
        "unsupported shape for hardcoded sharding"
    )

    fast, co = _coefficients(mean, cov, const)
    nc = _build_cached(W, CW, fast, co)

    in_maps = [
        {"x": tensor[i * per : (i + 1) * per].reshape(P, W)} for i in range(N_CORES)
    ]
    try:
        res = bass_utils.run_bass_kernel_spmd(
            nc,
            in_maps,
            core_ids=list(range(N_CORES)),
            trace=TRACE,
            **TRACE_KWARGS,
        )
    except ModuleNotFoundError:
        # NTFF profiling hook (antenv.axon_hooks) absent in this container;
        # rerun without tracing.
        res = bass_utils.run_bass_kernel_spmd(
            nc, in_maps, core_ids=list(range(N_CORES)), trace=False
        )
    LAST_RESULTS = res
    out = np.concatenate(
        [
            np.asarray(res.results[i]["y"]).reshape(-1).astype(np.float32)
            for i in range(N_CORES)
        ]
    )
    return out
